# revision 9
# baseline (speedup 1.0000x reference)
"""Depthwise 4x4 blur (upfirdn2d pad=(2,1)) on TRN2, 8 NeuronCores.

Math: out[h,w] = sum_{i,j} Kf[i,j] * x[h+i-2, w+j-2]   (Kf = flipped 2D kernel,
out-of-range terms = zero padding). For each kernel column j this is a banded
128x128 matrix A_j applied over H to a W-shifted slice of the padded image:

    OUT = sum_j A_j @ Xpad[:, j:j+128]      (PSUM accumulation over j)

so one image needs 4 TensorE matmuls and no transposes. H-padding is folded
into the band clipping of A_j; W-padding is baked into the host-side layout
(stride-131 rows: [0, 0, x0..x127, 0]). Sharding: batch dim (8 batches ->
8 cores), each core processes 256 images of 128x128.

float32r notes (measured on HW): matmul operands tagged float32r run the PE
at full rate (1 col/cycle vs 4 for float32) and are bit-exact when operands
have <= 11 mantissa bits; full-mantissa operands see ~1.3e-4 relative error.
Blur weights are exact in a few bits, so a hi/lo split of x recovers fp32
accuracy while keeping the fast path.
"""

import numpy as np
from contextlib import ExitStack

import concourse.bass as bass
import concourse.bacc as bacc
import concourse.tile as tile
import concourse.mybir as mybir
from concourse.bass_utils import run_bass_kernel_spmd

N_CORES = 8
B, C, H, W = 8, 256, 128, 128
WP = W + 3         # padded image stride: [0, 0, x0..x127, 0]
GROUP = 4          # images per PSUM bank (4*128 = 512 f32 = one bank)
SUPER = 16         # images per DMA (~1 MB transfers)
MM_DT = mybir.dt.float32r


def _body(ctx, tc, o_ap, x_ap, w_ap, mm_dt):
    nc = tc.nc
    wpool = ctx.enter_context(tc.tile_pool(name="wts", bufs=1))
    xpool = ctx.enter_context(tc.tile_pool(name="xin", bufs=5))
    opool = ctx.enter_context(tc.tile_pool(name="oup", bufs=5))
    ppool = ctx.enter_context(tc.tile_pool(name="ps", bufs=6, space="PSUM"))

    wt = wpool.tile([H, 4 * H], mm_dt)
    nc.sync.dma_start(
        wt[:].rearrange("k (j m) -> k j m", j=4), w_ap.rearrange("j k m -> k j m")
    )

    for s in range(C // SUPER):
        xt = xpool.tile([H, SUPER * WP], mm_dt)
        xt3 = xt[:].rearrange("h (c w) -> h c w", c=SUPER)
        nc.sync.dma_start(
            xt3, x_ap[s * SUPER : (s + 1) * SUPER].rearrange("c h w -> h c w")
        )
        ot = opool.tile([H, SUPER * W], mybir.dt.float32)
        for g in range(SUPER // GROUP):
            pt = ppool.tile([H, GROUP * W], mybir.dt.float32)
            for j in range(4):
                lhsT = wt[:, j * H : (j + 1) * H]
                # 4 images, width-128 window at padded-col j: AP [[WP,4],[1,128]]
                rhs = xt3[:, g * GROUP : (g + 1) * GROUP, j : j + W]
                nc.tensor.matmul(
                    pt[:],
                    lhsT,
                    rhs,
                    start=(j == 0),
                    stop=(j == 3),
                )
            nc.vector.tensor_copy(ot[:, g * GROUP * W : (g + 1) * GROUP * W], pt[:])
        # output DMA on the ACT HWDGE ring; input stays on the SP ring
        nc.scalar.dma_start(
            o_ap[s * SUPER : (s + 1) * SUPER].rearrange("c h w -> h c w"),
            ot[:].rearrange("h (c w) -> h c w", c=SUPER),
        )


def build_module(mm_dt=MM_DT):
    nc = bacc.Bacc(
        "TRN2", target_bir_lowering=False, debug=False, num_devices=N_CORES
    )
    x_ap = nc.dram_tensor("x", [C, H, WP], mm_dt, kind="ExternalInput").ap()
    w_ap = nc.dram_tensor("wts", [4, H, H], mm_dt, kind="ExternalInput").ap()
    o_ap = nc.dram_tensor(
        "out", [C, H, W], mybir.dt.float32, kind="ExternalOutput"
    ).ap()
    with tile.TileContext(nc) as tc:
        with ExitStack() as ctx:
            _body(ctx, tc, o_ap, x_ap, w_ap, mm_dt)
    nc.compile()
    return nc


def band_mats(k2d):
    """WT[j] = A_j^T where A_j[h, h+i-2] = Kf[i, j] (rows clipped to [0,128))."""
    kf = np.asarray(k2d, np.float32)[::-1, ::-1]
    wts = np.zeros((4, H, H), np.float32)
    for j in range(4):
        for i in range(4):
            d = i - 2  # diagonal offset m - h
            h0, h1 = max(0, -d), min(H, H - d)
            idx = np.arange(h0, h1)
            wts[j, idx + d, idx] = kf[i, j]
    return wts


def pad_w(x_core):
    """[C,H,W] f32 -> [C,H,WP] with zero cols at 0,1 and WP-1."""
    xp = np.zeros((x_core.shape[0], H, WP), np.float32)
    xp[:, :, 2 : 2 + W] = x_core
    return xp


_module_cache = {}


def _get_module(mm_dt=MM_DT):
    if mm_dt not in _module_cache:
        _module_cache[mm_dt] = build_module(mm_dt)
    return _module_cache[mm_dt]


def kernel(x, kernel, _trace=False, _trace_kwargs=None, _mm_dt=None):
    x = np.asarray(x, np.float32)
    assert x.shape == (B, C, H, W), x.shape
    wts = band_mats(kernel)
    nc = _get_module(_mm_dt or MM_DT)
    in_maps = [{"x": pad_w(x[i]), "wts": wts} for i in range(N_CORES)]
    res = run_bass_kernel_spmd(
        nc, in_maps, list(range(N_CORES)), trace=_trace, **(_trace_kwargs or {})
    )
    out = np.stack([res.results[i]["out"] for i in range(N_CORES)], axis=0)
    if _trace:
        return out, res
    return out
